# revision 11
# baseline (speedup 1.0000x reference)
"""Trainium2 Bass kernel for CustomMultiHeadAttentionLayer (v13, streamed).

Reference computation (B=4, S=2048, D=512, H=8, hd=64):
    Q = query @ Wq.T + bq ; K = key @ Wk.T + bk ; V = value @ Wv.T + bv
    per head: P = softmax(Q K^T / 8) ; ctx = P V
    out = gelu(ctx, exact erf) @ Wo.T + bo

Sharding: 8 cores = 4 batches x 2 query-halves. Each core handles the full
key/value of one batch and 1024 query rows. No collectives.

All matmuls f32r (self-loading weights, 1.0 cyc/row at N>=512 — bf16 would
be no faster and costs a separate Ldweights SEQ slot per matmul).

v13 vs v9 (both use the same per-head math):
  - scores^T tile = K_h^T.T @ Q_h^T ([128k, 512q] psum), exp on ACT with
    scale=1/8 (scores bounded ~|10|, no max subtraction), ctx^T' accumulated
    over 16 k-tiles via [V_h|1].T @ exp(S^T): rows 0:64 ctx^T, row 64 the
    softmax denominator l; PE ones-broadcast + reciprocal + mul normalize.
  - software pipeline: scores of unit u+1 are emitted BEFORE ctx of unit
    u, so the in-order PE queue never head-of-line blocks on the exp the
    ctx depends on.
  - K/V projection of k-slice s is emitted interleaved into attention
    units that only touch slices < s (wave-diagonal streaming), hiding
    projection PE/DVE work under the ACT-bound attention stream.
  - attention waves (qs, head-pair) accumulate ctx in 2 psum banks across
    all 16 k-tiles; score psum is 2x[128,1024] double-buffered; the shared
    2-bank "big" ring serves projection / transpose strips / out-proj.
"""

import numpy as np
from contextlib import ExitStack

import concourse.bass as bass
import concourse.tile as tile
from concourse import bacc, mybir
from concourse.bass_utils import run_bass_kernel_spmd

P = 128
D = 512
H = 8
HD = 64
F32 = mybir.dt.float32
F32R = mybir.dt.float32r

ActF = mybir.ActivationFunctionType


def _make_pools(ctx, tc):
    pools = {}
    pools["consts"] = ctx.enter_context(tc.tile_pool(name="consts", bufs=1))
    pools["natb"] = ctx.enter_context(tc.tile_pool(name="natb", bufs=2))
    pools["rawT"] = ctx.enter_context(tc.tile_pool(name="rawT", bufs=2))
    pools["ptp"] = ctx.enter_context(tc.tile_pool(name="ptp", bufs=2))
    pools["ctxp"] = ctx.enter_context(tc.tile_pool(name="ctxp", bufs=2))
    pools["brp"] = ctx.enter_context(tc.tile_pool(name="brp", bufs=1))
    pools["outp"] = ctx.enter_context(tc.tile_pool(name="outp", bufs=2))
    pools["gp"] = ctx.enter_context(tc.tile_pool(name="gp", bufs=1))
    pools["persist"] = ctx.enter_context(tc.tile_pool(name="persist", bufs=1))
    pools["psum"] = ctx.enter_context(tc.tile_pool(name="psum", bufs=1, space="PSUM"))
    return pools


def _body(pools, tc, t, sq, sk, use_gelu=True):
    nc = tc.nc
    NQS = sq // 512          # 512-wide q slices (2)
    NKS = sk // 512          # 512-wide k slices (4)
    NKT = sk // P            # 128-wide k tiles (16)
    NQC = sq // P            # 128-wide q chunks (8)

    consts = pools["consts"]
    natb = pools["natb"]
    rawT = pools["rawT"]
    ptp = pools["ptp"]
    ctxp = pools["ctxp"]
    brp = pools["brp"]
    outp = pools["outp"]
    gp = pools["gp"]
    persist = pools["persist"]
    psum = pools["psum"]

    def ps_score(nm):
        # [128,1024] f32 = 2 banks; x2 bufs => 4 banks total
        return psum.tile([P, 1024], F32, name=nm, tag="score2", bufs=2)

    def ps_big(nm, dt=F32):
        # shared 2-bank ring: proj psum, transpose strips, l-broadcast, out-proj
        return psum.tile([P, 512], dt, name=nm, tag="big", bufs=2)

    def ps_acc(nm, tag):
        # per-wave ctx^T accumulators (rows 0:65 used), 1 bank each
        return psum.tile([P, 512], F32, name=nm, tag=tag, bufs=1)

    # ---------------- constants ----------------
    identz = consts.tile([P, 136], F32R, name="identz", tag="identz")
    nc.sync.dma_start(out=identz, in_=t["ident_in"][:, :])
    ident = identz[:, 0:P]

    def load512(src_rows, name):
        # ONE dma_start for 512 consecutive DRAM rows: row j*128+p lands at
        # partition p, cols [j*512:(j+1)*512)  (SP.SEQ DMA-issue is a serial
        # resource; 1 big DMA instead of 4 per slice)
        xb = natb.tile([P, 4 * D], F32R, name=name, tag="natb")
        nc.sync.dma_start(out=xb, in_=src_rows.rearrange("(j p) d -> p j d", p=P))
        return xb

    # ---------------- weight transposes ----------------
    wT = {}
    for w in ("wq", "wk"):
        wT[w] = [
            persist.tile([P, D], F32R, name=f"{w}T{m}", tag=f"{w}T{m}")
            for m in range(4)
        ]
    wT["wv"] = [
        persist.tile([P, 520], F32R, name=f"wvT{m}", tag=f"wvT{m}") for m in range(4)
    ]
    for m in range(4):
        for h in range(H):
            nc.vector.tensor_copy(
                out=wT["wv"][m][:, 65 * h + 64:65 * h + 65],
                in_=identz[:, P + h:P + h + 1],
            )

    for w in ("wq", "wk", "wv"):
        wb = load512(t[w][0:D, :], f"{w}nat")
        for m in range(4):  # d_in chunk
            pt = ps_big("trp", F32R)
            for j in range(4):
                nc.tensor.transpose(
                    pt[:, j * P:(j + 1) * P],
                    wb[:, j * D + m * P:j * D + (m + 1) * P], ident
                )
            if w == "wv":
                for hh in range(H):
                    nc.vector.tensor_copy(
                        out=wT[w][m][:, 65 * hh:65 * hh + 64],
                        in_=pt[:, 64 * hh:64 * hh + 64],
                    )
            else:
                nc.vector.tensor_copy(out=wT[w][m], in_=pt)

    # Wo^T per head [64, 512] at base partition 0 (matches g0 lhsT base)
    woTh = [
        persist.tile([64, D], F32R, name=f"woTh{h}", tag=f"woTh{h}")
        for h in range(H)
    ]
    wob = load512(t["wo"][0:D, :], "wonat")
    for h in range(H):
        pt = ps_big("trpo", F32R)
        for j in range(4):
            nc.tensor.transpose(
                pt[0:64, j * P:(j + 1) * P],
                wob[:, j * D + h * HD:j * D + (h + 1) * HD], ident
            )
        nc.vector.tensor_copy(out=woTh[h], in_=pt[0:64, :])

    ones65 = consts.tile([65, P], F32R, name="ones65", tag="ones65")
    nc.sync.dma_start(out=ones65, in_=t["ones_in"][:, :])

    bqk = consts.tile([P, 8], F32, name="bqk", tag="bqk")
    nc.sync.dma_start(out=bqk[:, 0:4], in_=t["bq"][:].rearrange("(c p) -> p c", p=P))
    nc.sync.dma_start(out=bqk[:, 4:8], in_=t["bk"][:].rearrange("(c p) -> p c", p=P))

    bvb = consts.tile([P, 520], F32, name="bvb", tag="bvb")
    for h in range(H):
        src = t["bv"][h * HD:(h + 1) * HD]
        bsrc = bass.AP(tensor=src.tensor, offset=src.offset, ap=[[0, P]] + src.ap)
        nc.sync.dma_start(out=bvb[:, 65 * h:65 * h + 64], in_=bsrc)
        nc.gpsimd.memset(bvb[:, 65 * h + 64:65 * h + 65], 1.0)

    bob = consts.tile([P, D], F32, name="bob", tag="bob")
    bo_ap = t["bo"][:]
    nc.sync.dma_start(
        out=bob,
        in_=bass.AP(tensor=bo_ap.tensor, offset=bo_ap.offset, ap=[[0, P]] + bo_ap.ap),
    )

    # ---------------- persistent activation tiles ----------------
    QT = [persist.tile([P, sq], F32R, name=f"QT{m}", tag=f"QT{m}") for m in range(4)]
    KT = [persist.tile([P, sk], F32R, name=f"KT{m}", tag=f"KT{m}") for m in range(4)]
    Vp = [
        persist.tile([P, 520], F32R, name=f"Vp{kt}", tag=f"Vp{kt}")
        for kt in range(NKT)
    ]
    g0 = [gp.tile([64, sq], F32R, name=f"g0_{h}", tag=f"g0_{h}") for h in range(H)]

    # ---------------- projection chunk builders ----------------
    def strip_transpose(xb, i, strip_name):
        pt = ps_big("trs", F32R)
        for j in range(4):
            nc.tensor.transpose(
                pt[:, j * P:(j + 1) * P],
                xb[:, j * D + i * P:j * D + (i + 1) * P], ident
            )
        xT = rawT.tile([P, 512], F32R, name=strip_name, tag=strip_name, bufs=1)
        nc.vector.tensor_copy(out=xT, in_=pt)
        return xT

    def qk_chunks(src, s, which):
        st = {"xb": None, "xT": [None] * 4}
        dst, bcol = (QT, 0) if which == "q" else (KT, 4)
        wkey = "wq" if which == "q" else "wk"

        def dma():
            st["xb"] = load512(src[s * 512:(s + 1) * 512, :], "xnat")

        def mk_strip(i):
            def f():
                st["xT"][i] = strip_transpose(st["xb"], i, f"xT{i}")
            return f

        def mk_proj(m):
            def f():
                pk = ps_big("pk")
                for i in range(4):
                    nc.tensor.matmul(
                        pk, wT[wkey][i][:, m * P:(m + 1) * P], st["xT"][i],
                        start=(i == 0), stop=(i == 3),
                    )
                nc.vector.tensor_scalar_add(
                    out=dst[m][:, s * 512:(s + 1) * 512],
                    in0=pk,
                    scalar1=bqk[:, bcol + m:bcol + m + 1],
                )
            return f

        return [dma] + [mk_strip(i) for i in range(4)] + [mk_proj(m) for m in range(4)]

    def v_chunks(s):
        st = {"xb": None, "xT": [None] * 4}

        def dma():
            st["xb"] = load512(t["v_in"][s * 512:(s + 1) * 512, :], "xnat")

        def mk_strip(i):
            def f():
                st["xT"][i] = strip_transpose(st["xb"], i, f"xT{i}")
            return f

        def mk_proj(j):
            def f():
                kt = s * 4 + j
                pva = ps_big("pva")
                pvb = ps_big("pvb")
                for i in range(4):
                    nc.tensor.matmul(
                        pva[:, 0:260],
                        st["xT"][i][:, j * P:(j + 1) * P],
                        wT["wv"][i][:, 0:260],
                        start=(i == 0), stop=(i == 3),
                    )
                for i in range(4):
                    nc.tensor.matmul(
                        pvb[:, 0:260],
                        st["xT"][i][:, j * P:(j + 1) * P],
                        wT["wv"][i][:, 260:520],
                        start=(i == 0), stop=(i == 3),
                    )
                nc.vector.tensor_add(
                    out=Vp[kt][:, 0:260], in0=pva[:, 0:260], in1=bvb[:, 0:260]
                )
                nc.vector.tensor_add(
                    out=Vp[kt][:, 260:520], in0=pvb[:, 0:260], in1=bvb[:, 260:520]
                )
            return f

        return [dma] + [mk_strip(i) for i in range(4)] + [mk_proj(j) for j in range(4)]

    # ---------------- phase 0: Q half 0 + k/v slice 0 ----------------
    for c in qk_chunks(t["q_in"], 0, "q"):
        c()
    for c in qk_chunks(t["k_in"], 0, "k") + v_chunks(0):
        c()

    # chunk queue: (ready_slice, fn) — chunk must be emitted before any
    # attention unit that consumes k-slice >= ready_slice
    queue = []
    if NQS > 1:
        queue += [(1, c) for c in qk_chunks(t["q_in"], 1, "q")]
    for s in range(1, NKS):
        queue += [(s, c) for c in qk_chunks(t["k_in"], s, "k") + v_chunks(s)]

    # ---------------- attention: waves x k, wave-diagonal streaming -------
    def emit_S(qs, hp, kt2):
        psc = [ps_score("psc0"), ps_score("psc1")]
        for g in range(2):
            kt = 2 * kt2 + g
            for s2 in range(2):
                nc.tensor.matmul(
                    psc[s2][:, g * 512:(g + 1) * 512],
                    KT[hp][64 * s2:64 * s2 + 64, kt * P:(kt + 1) * P],
                    QT[hp][64 * s2:64 * s2 + 64, qs * 512:(qs + 1) * 512],
                    start=True, stop=True,
                )
        pts = []
        for s2 in range(2):
            pT = ptp.tile([P, 1024], F32R, name=f"pT{s2}", tag=f"pT{s2}", bufs=2)
            nc.scalar.activation(pT, psc[s2], ActF.Exp, scale=0.125)
            pts.append(pT)
        return pts

    def emit_C(accs, hp, kt2, pts):
        for s2 in range(2):
            h = 2 * hp + s2
            for g in range(2):
                kt = 2 * kt2 + g
                nc.tensor.matmul(
                    accs[s2][0:65, :],
                    Vp[kt][:, 65 * h:65 * h + 65],
                    pts[s2][:, g * 512:(g + 1) * 512],
                    start=(kt == 0), stop=(kt == NKT - 1),
                )

    def flush(upto_slice, budget):
        n = 0
        while queue and (queue[0][0] <= upto_slice or n < budget):
            queue.pop(0)[1]()
            n += 1

    NU = NKT // 2  # kt2 units per wave (8)
    total_units = NQS * 4 * NU
    pace = (len(queue) + total_units - 1) // total_units + 1
    for qs in range(NQS):
        for hp in range(4):
            accs = (ps_acc("accA", "accA"), ps_acc("accB", "accB"))
            prev = None
            for kt2 in range(NU):
                flush(kt2 // 2, 0)
                pts = emit_S(qs, hp, kt2)
                if prev is not None:
                    emit_C(accs, hp, prev[0], prev[1])
                flush(-1, pace)
                prev = (kt2, pts)
            emit_C(accs, hp, prev[0], prev[1])
            # normalize: l broadcast (PE), reciprocal + mul (DVE) -> g0
            for s2 in range(2):
                h = 2 * hp + s2
                csb = ctxp.tile([65, 512], F32R, name="csb", tag="csb")
                nc.vector.tensor_copy(out=csb, in_=accs[s2][0:65, :])
                pb = ps_big("pb")
                nc.tensor.matmul(pb, ones65[64:65, :], csb[64:65, :],
                                 start=True, stop=True)
                brec = brp.tile([P, 512], F32, name="brec", tag="brec")
                nc.vector.reciprocal(out=brec, in_=pb)
                nc.vector.tensor_mul(
                    out=g0[h][:, qs * 512:(qs + 1) * 512],
                    in0=csb[0:64, :],
                    in1=brec[0:64, :],
                )

    # ---------------- epilogue: gelu + output projection ----------------
    gelu_f = ActF.Gelu if use_gelu else ActF.Identity
    for h in range(H):
        nc.scalar.activation(g0[h], g0[h], gelu_f)

    for qc in range(NQC):
        po = ps_big("po")
        for h in range(H):
            nc.tensor.matmul(
                po,
                g0[h][:, qc * P:(qc + 1) * P],
                woTh[h],
                start=(h == 0), stop=(h == H - 1),
            )
        osb = outp.tile([P, D], F32, name="osb", tag="osb")
        nc.vector.tensor_add(out=osb, in0=po, in1=bob)
        nc.sync.dma_start(out=t["out"][qc * P:(qc + 1) * P, :], in_=osb)


def build(sq=1024, sk=2048, use_gelu=True, bench_iters=1):
    nc = bacc.Bacc(None)
    t = {}
    t["q_in"] = nc.dram_tensor("q_in", [sq, D], F32R, kind="ExternalInput")
    t["k_in"] = nc.dram_tensor("k_in", [sk, D], F32R, kind="ExternalInput")
    t["v_in"] = nc.dram_tensor("v_in", [sk, D], F32R, kind="ExternalInput")
    for w in ("wq", "wk", "wv", "wo"):
        t[w] = nc.dram_tensor(w, [D, D], F32R, kind="ExternalInput")
    for b in ("bq", "bk", "bv", "bo"):
        t[b] = nc.dram_tensor(b, [D], F32, kind="ExternalInput")
    t["ident_in"] = nc.dram_tensor("ident_in", [P, 136], F32R, kind="ExternalInput")
    t["ones_in"] = nc.dram_tensor("ones_in", [65, P], F32R, kind="ExternalInput")
    t["out"] = nc.dram_tensor("out", [sq, D], F32, kind="ExternalOutput")

    with ExitStack() as ctx:
        tc = ctx.enter_context(tile.TileContext(nc))
        pools = _make_pools(ctx, tc)
        if bench_iters > 1:
            with tc.For_i(0, bench_iters, 1):
                _body(pools, tc, t, sq, sk, use_gelu=use_gelu)
        else:
            _body(pools, tc, t, sq, sk, use_gelu=use_gelu)
    if not nc.is_finalized():
        nc.finalize()
    return nc


_NC_CACHE = {}


def _get_nc(sq, sk):
    key = (sq, sk)
    if key not in _NC_CACHE:
        _NC_CACHE[key] = build(sq, sk)
    return _NC_CACHE[key]


def make_in_maps(query, key, value, Wq, bq, Wk, bk, Wv, bv, Wo, bo):
    B, SQ, _ = query.shape
    half = SQ // 2
    f = np.ascontiguousarray
    ident_in = np.zeros((128, 136), np.float32)
    ident_in[:, :128] = np.eye(128, dtype=np.float32)
    ones_in = np.ones((65, 128), np.float32)
    in_maps = []
    for c in range(8):
        b, qh = c // 2, c % 2
        in_maps.append({
            "ident_in": ident_in,
            "ones_in": ones_in,
            "q_in": f(query[b, qh * half:(qh + 1) * half]).astype(np.float32),
            "k_in": f(key[b]).astype(np.float32),
            "v_in": f(value[b]).astype(np.float32),
            "wq": f(Wq).astype(np.float32),
            "wk": f(Wk).astype(np.float32),
            "wv": f(Wv).astype(np.float32),
            "wo": f(Wo).astype(np.float32),
            "bq": f(bq).astype(np.float32),
            "bk": f(bk).astype(np.float32),
            "bv": f(bv).astype(np.float32),
            "bo": f(bo).astype(np.float32),
        })
    return in_maps


def kernel(query, key, value, Wq, bq, Wk, bk, Wv, bv, Wo, bo, **run_kwargs):
    query = np.asarray(query)
    B, SQ, _ = query.shape
    half = SQ // 2
    nc = _get_nc(half, np.asarray(key).shape[1])
    in_maps = make_in_maps(
        query, np.asarray(key), np.asarray(value),
        np.asarray(Wq), np.asarray(bq), np.asarray(Wk), np.asarray(bk),
        np.asarray(Wv), np.asarray(bv), np.asarray(Wo), np.asarray(bo),
    )
    res = run_bass_kernel_spmd(nc, in_maps, core_ids=list(range(8)), **run_kwargs)
    out = np.empty((B, SQ, D), np.float32)
    for c in range(8):
        b, qh = c // 2, c % 2
        out[b, qh * half:(qh + 1) * half] = res.results[c]["out"]
    kernel.last_results = res
    return out
